# revision 20
# baseline (speedup 1.0000x reference)
"""Trainium2 Bass kernel for the ActorCriticCriterion (AIC) masked REINFORCE loss.

Reference computation (per the oracle):
    at_or_after_eos = cumsum(seq == 0, axis=1) > 0
    seq_z  = where(at_or_after_eos, 0, seq)
    mask   = concat([ones(B,1), (seq_z > 0)[:, :-1]], axis=1)
    loss   = sum(-logp * (reward - value) * mask) / sum(mask)

Identity used: mask[t] = AND(seq[0..t-1] != 0) with mask[0] = 1 — computed
directly with one DVE tensor_tensor_scan (op0=logical_and) per 128-row group,
writing to a shifted access pattern (the leading ones column is a memset).

fp16, two-phase pipeline (streaming is the roofline — ~41us of HBM traffic at
the ~400-410 GB/s/core measured rate; all compute must hide under it):
    DMA:    single sync HWDGE ring, fully pre-issuable (staging is fully
            resident, so the stream never stalls on a buffer).  All seq tiles
            stream FIRST, then per-sub-block val/rew/lp groups.  The final
            tile of the stream is the last sub-block's lp, whose remaining
            work is exactly one DVE op.
    Phase 1 (hides under the seq stream): all masks via logical_and scans
            (fp16 out; scans have no 2x mode) + den matmuls.  The den
            accumulator closes ~20us before the stream ends, so its copy and
            store are entirely off the critical path.
    Phase 2: casts on the Activation engine (fp16 operands make DVE
            tensor_tensor 2x and matmuls 1 cycle/row), then per sub-block
            d = val16 - rew16, dm = d * mask, mq = lp16 * dm, num matmuls.
            The last sub-block skips the casts: its d/mq read f32 at 1x so
            no scalar hop sits on the stream-end -> output critical path.
    PE:     ones16[128,1].T @ {mask,mq} chunks accumulated into two
            single-bank PSUM accumulators num/den [1, 512] f32.
Outputs are the two [1, 512] accumulators; the host sums them and divides.
Sharding: pure data-parallel over B across 8 cores (1024 rows each).

Hard-won constraints (measured):
  - Never slice the DRAM side of a dma_start along T: a strided DRAM source
    defeats descriptor coalescing and runs at ~25 GB/s.
  - Only whole-tile SBUF writes: region-sliced tile writes have shown
    dropped-dependency races.
  - GpSimd must stay idle (shared SBUF port with DVE), and it cannot touch
    PSUM at all on trn2.
"""

import os
import numpy as np

B, T = 8192, 1024
NCORES = 8
ROWS = B // NCORES          # rows per core
P = 128                     # SBUF partitions
MMCHUNK = 512               # matmul free-dim chunk (one PSUM bank)

_CACHE: dict = {}


def _build_program(rows: int):
    """Build the Bass/Tile program for one core processing `rows` rows."""
    from contextlib import ExitStack

    import concourse.bacc as bacc
    import concourse.mybir as mybir
    import concourse.tile as tile

    f32 = mybir.dt.float32
    f16 = mybir.dt.float16
    i32 = mybir.dt.int32
    Op = mybir.AluOpType

    ablk = int(os.environ.get("K_A", "1"))     # row-groups per sub-block
    nsub = rows // (P * ablk)
    assert nsub * P * ablk == rows

    # Bacc (not raw Bass): its compile pipeline splits multi-sem sync waits
    # into event-semaphore instructions — this walrus build allows at most
    # one wait per engine instruction.
    nc = bacc.Bacc()
    seq = nc.dram_tensor("seq", [rows, T], i32, kind="ExternalInput")
    lp = nc.dram_tensor("lp", [rows, T], f32, kind="ExternalInput")
    val = nc.dram_tensor("val", [rows, T], f32, kind="ExternalInput")
    rew = nc.dram_tensor("rew", [rows, T], f32, kind="ExternalInput")
    out_num = nc.dram_tensor("out_num", [1, MMCHUNK], f32,
                             kind="ExternalOutput")
    out_den = nc.dram_tensor("out_den", [1, MMCHUNK], f32,
                             kind="ExternalOutput")

    def dram_sub(t, r0, na):
        # rows [r0, r0 + na*P) as [p, a, t] with row = r0 + a*P + p
        return t[r0:r0 + na * P, :].rearrange("(a p) t -> p a t", p=P)

    light_tail = bool(int(os.environ.get("K_LIGHT_TAIL", "1")))

    with ExitStack() as ctx:
        tc = ctx.enter_context(tile.TileContext(nc))
        if light_tail:
            # Replace Tile's end-of-kernel epilogue (drain + two all-engine
            # EVSEM barriers + 64-sem clear, ~8-9us) with just the final
            # drain. Safe for re-execution: the Bass preamble dma_reset +
            # sem_clear runs at the START of every execution, so leaving
            # semaphores dirty at kernel end is fine.
            import types

            from concourse.vector_clock import ScopedClock

            def _light_drain_and_barrier(self, tick_clock, wait_clock):
                drain_inst = self.nc.sync.drain()
                wait_clock.add_sem_waits(
                    drain_inst.ins,
                    ScopedClock({None: tick_clock.global_clock}))
                popped = self.nc._tile_sem_poison_stack.pop()
                assert popped is self._sem_poison
                # Deliberately do NOT free the tile sems: Bacc's
                # event-semaphore pass allocates from the free pool after
                # this and must not alias sems still used by the kernel.

            tc._drain_and_barrier = types.MethodType(
                _light_drain_and_barrier, tc)
        const_pool = ctx.enter_context(tc.tile_pool(name="const", bufs=1))
        # One staging buffer per sub-block: the DMA ring never waits on a
        # buffer free, so the stream runs gap-free at line rate.
        in_pool = ctx.enter_context(tc.tile_pool(name="in", bufs=nsub))
        h_pool = ctx.enter_context(tc.tile_pool(name="h", bufs=2))
        scr_pool = ctx.enter_context(tc.tile_pool(name="scr", bufs=2))
        psum_pool = ctx.enter_context(
            tc.tile_pool(name="psum", bufs=1, space="PSUM"))

        ones16 = const_pool.tile([P, 1], f16)
        nc.vector.memset(ones16[:], 1.0)

        num_ps = psum_pool.tile([1, MMCHUNK], f32)
        den_ps = psum_pool.tile([1, MMCHUNK], f32)

        na = ablk

        # ---- DMA pre-issue (single ring; issue order = arrival order):
        # all seq tiles, then the LAST sub-block's val/rew (so its d/dm can
        # be computed ~20us early), then the val/rew/lp groups, and the last
        # sub-block's lp as the very final tile — its only remaining work is
        # the single mq op.
        seq_ts, lp_ts, val_ts, rew_ts = [], [], [], [None] * nsub
        val_ts = [None] * nsub
        lp_ts = [None] * nsub
        rew_ts = [None] * nsub
        for si in range(nsub):
            r0 = si * P * na
            seq_t = in_pool.tile([P, na, T], i32, tag="seq")
            nc.sync.dma_start(out=seq_t[:], in_=dram_sub(seq, r0, na))
            seq_ts.append(seq_t)
        li = nsub - 1

        # Optional second HWDGE ring (the ACT engine's): rew+lp stream there,
        # pre-issued ahead of all the casts on the same queue.
        ring2 = nc.scalar if bool(int(os.environ.get("K_RING_SPLIT", "0"))) \
            else nc.sync

        def issue_vr(si):
            r0 = si * P * na
            # NOTE: never slice the DRAM side along T — a strided DRAM
            # source defeats descriptor coalescing (~25 GB/s measured).
            val_ts[si] = in_pool.tile([P, na, T], f32, tag="val", name=f"val{si}")
            rew_ts[si] = in_pool.tile([P, na, T], f32, tag="rew", name=f"rew{si}")
            nc.sync.dma_start(out=val_ts[si][:], in_=dram_sub(val, r0, na))
            ring2.dma_start(out=rew_ts[si][:], in_=dram_sub(rew, r0, na))

        def issue_lp(si):
            r0 = si * P * na
            lp_ts[si] = in_pool.tile([P, na, T], f32, tag="lp", name=f"lp{si}")
            ring2.dma_start(out=lp_ts[si][:], in_=dram_sub(lp, r0, na))

        if nsub > 1:
            issue_vr(li)
        for si in range(nsub - 1):
            issue_vr(si)
            issue_lp(si)
        if nsub == 1:
            issue_vr(li)
        issue_lp(li)

        # ---- Phase 1: masks (scans) + den matmuls, hidden under the stream.
        masks = []
        for si in range(nsub):
            seq_t = seq_ts[si]
            mask = scr_pool.tile([P, na, T], f16, tag="mask", bufs=nsub)
            nc.vector.memset(mask[:, :, 0:1], 1.0)
            for a in range(na):
                nc.vector.tensor_tensor_scan(
                    out=mask[:, a, 1:T], data0=seq_t[:, a, 0:T - 1],
                    data1=seq_t[:, a, 0:T - 1], initial=1.0,
                    op0=Op.logical_and, op1=Op.bypass)
            for a in range(na):
                for c in range(0, T, MMCHUNK):
                    nc.tensor.matmul(
                        out=den_ps[:], lhsT=ones16[:],
                        rhs=mask[:, a, c:c + MMCHUNK],
                        start=(si == 0 and a == 0 and c == 0),
                        stop=(si == nsub - 1 and a == na - 1
                              and c == T - MMCHUNK))
            masks.append(mask)

        # ---- Phase 2: casts + d/dm/mq + num matmuls.
        # The last sub-block's d/dm are emitted FIRST (its val/rew streamed
        # right after the seqs, and it skips the casts — f32 at 1x), so that
        # when its lp lands as the stream's final tile, the one remaining op
        # is mq.  dm (not q=lp*d) is the grouping that makes the lp-
        # dependent work a single op.
        d_last = scr_pool.tile([P, na, T], f16, tag="d_last", bufs=1)
        nc.vector.tensor_tensor(out=d_last[:], in0=val_ts[li][:],
                                in1=rew_ts[li][:], op=Op.subtract)
        dm_last = scr_pool.tile([P, na, T], f16, tag="dm_last", bufs=1)
        nc.vector.tensor_tensor(out=dm_last[:], in0=d_last[:],
                                in1=masks[li][:], op=Op.mult)

        for si in range(nsub - 1):
            lp_t, val_t, rew_t = lp_ts[si], val_ts[si], rew_ts[si]
            mask = masks[si]
            lp16 = h_pool.tile([P, na, T], f16, tag="lp16")
            val16 = h_pool.tile([P, na, T], f16, tag="val16")
            rew16 = h_pool.tile([P, na, T], f16, tag="rew16")
            nc.scalar.copy(val16[:], val_t[:])
            nc.scalar.copy(rew16[:], rew_t[:])
            nc.scalar.copy(lp16[:], lp_t[:])

            d16 = scr_pool.tile([P, na, T], f16, tag="d")
            nc.vector.tensor_tensor(out=d16[:], in0=val16[:], in1=rew16[:],
                                    op=Op.subtract)
            dm = scr_pool.tile([P, na, T], f16, tag="dm")
            nc.vector.tensor_tensor(out=dm[:], in0=d16[:], in1=mask[:],
                                    op=Op.mult)
            mq = scr_pool.tile([P, na, T], f16, tag="mq", bufs=3)
            nc.vector.tensor_tensor(out=mq[:], in0=lp16[:], in1=dm[:],
                                    op=Op.mult)
            for a in range(na):
                for c in range(0, T, MMCHUNK):
                    nc.tensor.matmul(
                        out=num_ps[:], lhsT=ones16[:],
                        rhs=mq[:, a, c:c + MMCHUNK],
                        start=(si == 0 and a == 0 and c == 0),
                        stop=False)

        # Tail: the stream's final tile -> one TT -> two matmuls.
        mq_last = scr_pool.tile([P, na, T], f16, tag="mq_last", bufs=1)
        nc.vector.tensor_tensor(out=mq_last[:], in0=lp_ts[li][:],
                                in1=dm_last[:], op=Op.mult)
        for a in range(na):
            for c in range(0, T, MMCHUNK):
                nc.tensor.matmul(
                    out=num_ps[:], lhsT=ones16[:],
                    rhs=mq_last[:, a, c:c + MMCHUNK],
                    start=(nsub == 1 and a == 0 and c == 0),
                    stop=(a == na - 1 and c == T - MMCHUNK))

        # PSUM can't be DMA'd directly — bounce through SBUF.  den closed in
        # phase 1, so its copy + store fully overlap phase 2; num's copy is
        # split across the two free engines.
        num_sb = const_pool.tile([1, MMCHUNK], f32)
        den_sb = const_pool.tile([1, MMCHUNK], f32)
        nc.scalar.copy(den_sb[:], den_ps[:])
        nc.sync.dma_start(out=out_den[:], in_=den_sb[:])
        nc.vector.tensor_copy(num_sb[:], num_ps[:])
        nc.sync.dma_start(out=out_num[:], in_=num_sb[:])

    nc.finalize()
    return nc


def kernel(sample_seq, sample_seqLogprobs, sample_value, sample_reward):
    from concourse.bass_utils import run_bass_kernel_spmd

    seq = np.ascontiguousarray(np.asarray(sample_seq, dtype=np.int32))
    lp = np.ascontiguousarray(np.asarray(sample_seqLogprobs, dtype=np.float32))
    val = np.ascontiguousarray(np.asarray(sample_value, dtype=np.float32))
    rew = np.ascontiguousarray(np.asarray(sample_reward, dtype=np.float32))
    assert seq.shape == (B, T)

    if "nc" not in _CACHE:
        _CACHE["nc"] = _build_program(ROWS)
    nc = _CACHE["nc"]

    in_maps = []
    for c in range(NCORES):
        sl = slice(c * ROWS, (c + 1) * ROWS)
        in_maps.append({
            "seq": seq[sl], "lp": lp[sl], "val": val[sl], "rew": rew[sl],
        })

    trace = bool(int(os.environ.get("K_TRACE", "0")))
    res = run_bass_kernel_spmd(nc, in_maps, core_ids=list(range(NCORES)),
                               trace=trace)
    if trace:
        _CACHE["exec_time_ns"] = res.exec_time_ns
        _CACHE["trace"] = res.instructions_and_trace
    num = 0.0
    den = 0.0
    for r in res.results:
        num += float(np.asarray(r["out_num"], dtype=np.float64).sum())
        den += float(np.asarray(r["out_den"], dtype=np.float64).sum())
    return np.float32(num / den)


# revision 21
# speedup vs baseline: 1.0991x; 1.0991x over previous
"""Trainium2 Bass kernel for the ActorCriticCriterion (AIC) masked REINFORCE loss.

Reference computation (per the oracle):
    at_or_after_eos = cumsum(seq == 0, axis=1) > 0
    seq_z  = where(at_or_after_eos, 0, seq)
    mask   = concat([ones(B,1), (seq_z > 0)[:, :-1]], axis=1)
    loss   = sum(-logp * (reward - value) * mask) / sum(mask)

Identity used: mask[t] = AND(seq[0..t-1] != 0) with mask[0] = 1 — computed
directly with one DVE tensor_tensor_scan (op0=logical_and) per 128-row group,
writing to a shifted access pattern (the leading ones column is a memset).

fp16, two-phase pipeline (streaming is the roofline — ~41us of HBM traffic at
the ~400-410 GB/s/core measured rate; all compute must hide under it):
    DMA:    single sync HWDGE ring, fully pre-issuable (staging is fully
            resident, so the stream never stalls on a buffer).  All seq tiles
            stream FIRST, then per-sub-block val/rew/lp groups.  The final
            tile of the stream is the last sub-block's lp, whose remaining
            work is exactly one DVE op.
    Phase 1 (hides under the seq stream): all masks via logical_and scans
            (fp16 out; scans have no 2x mode) + den matmuls.  The den
            accumulator closes ~20us before the stream ends, so its copy and
            store are entirely off the critical path.
    Phase 2: casts on the Activation engine (fp16 operands make DVE
            tensor_tensor 2x and matmuls 1 cycle/row), then per sub-block
            d = val16 - rew16, dm = d * mask, mq = lp16 * dm, num matmuls.
            The last sub-block skips the casts: its d/mq read f32 at 1x so
            no scalar hop sits on the stream-end -> output critical path.
    PE:     ones16[128,1].T @ {mask,mq} chunks accumulated into two
            single-bank PSUM accumulators num/den [1, 512] f32.
Outputs are the two [1, 512] accumulators; the host sums them and divides.
Sharding: pure data-parallel over B across 8 cores (1024 rows each).

Hard-won constraints (measured):
  - Never slice the DRAM side of a dma_start along T: a strided DRAM source
    defeats descriptor coalescing and runs at ~25 GB/s.
  - Only whole-tile SBUF writes: region-sliced tile writes have shown
    dropped-dependency races.
  - GpSimd must stay idle (shared SBUF port with DVE), and it cannot touch
    PSUM at all on trn2.
"""

import os
import numpy as np

B, T = 8192, 1024
NCORES = 8
ROWS = B // NCORES          # rows per core
P = 128                     # SBUF partitions
MMCHUNK = 512               # matmul free-dim chunk (one PSUM bank)

_CACHE: dict = {}


def _build_program(rows: int):
    """Build the Bass/Tile program for one core processing `rows` rows."""
    from contextlib import ExitStack

    import concourse.bacc as bacc
    import concourse.mybir as mybir
    import concourse.tile as tile

    f32 = mybir.dt.float32
    f16 = mybir.dt.float16
    i32 = mybir.dt.int32
    Op = mybir.AluOpType

    ablk = int(os.environ.get("K_A", "1"))     # row-groups per sub-block
    nsub = rows // (P * ablk)
    assert nsub * P * ablk == rows

    # Bacc (not raw Bass): its compile pipeline splits multi-sem sync waits
    # into event-semaphore instructions — this walrus build allows at most
    # one wait per engine instruction.
    nc = bacc.Bacc()
    seq = nc.dram_tensor("seq", [rows, T], i32, kind="ExternalInput")
    lp = nc.dram_tensor("lp", [rows, T], f32, kind="ExternalInput")
    val = nc.dram_tensor("val", [rows, T], f32, kind="ExternalInput")
    rew = nc.dram_tensor("rew", [rows, T], f32, kind="ExternalInput")
    out_num = nc.dram_tensor("out_num", [1, MMCHUNK], f32,
                             kind="ExternalOutput")
    out_den = nc.dram_tensor("out_den", [1, MMCHUNK], f32,
                             kind="ExternalOutput")

    def dram_sub(t, r0, na):
        # rows [r0, r0 + na*P) as [p, a, t] with row = r0 + a*P + p
        return t[r0:r0 + na * P, :].rearrange("(a p) t -> p a t", p=P)

    light_tail = bool(int(os.environ.get("K_LIGHT_TAIL", "1")))

    with ExitStack() as ctx:
        tc = ctx.enter_context(tile.TileContext(nc))
        if light_tail:
            # Replace Tile's end-of-kernel epilogue (drain + two all-engine
            # EVSEM barriers + 64-sem clear, ~8-9us) with just the final
            # drain. Safe for re-execution: the Bass preamble dma_reset +
            # sem_clear runs at the START of every execution, so leaving
            # semaphores dirty at kernel end is fine.
            import types

            from concourse.vector_clock import ScopedClock

            def _light_drain_and_barrier(self, tick_clock, wait_clock):
                drain_inst = self.nc.sync.drain()
                wait_clock.add_sem_waits(
                    drain_inst.ins,
                    ScopedClock({None: tick_clock.global_clock}))
                popped = self.nc._tile_sem_poison_stack.pop()
                assert popped is self._sem_poison
                # Deliberately do NOT free the tile sems: Bacc's
                # event-semaphore pass allocates from the free pool after
                # this and must not alias sems still used by the kernel.

            tc._drain_and_barrier = types.MethodType(
                _light_drain_and_barrier, tc)
        const_pool = ctx.enter_context(tc.tile_pool(name="const", bufs=1))
        # One staging buffer per sub-block: the DMA ring never waits on a
        # buffer free, so the stream runs gap-free at line rate.
        in_pool = ctx.enter_context(tc.tile_pool(name="in", bufs=nsub))
        h_pool = ctx.enter_context(tc.tile_pool(name="h", bufs=2))
        scr_pool = ctx.enter_context(tc.tile_pool(name="scr", bufs=2))
        psum_pool = ctx.enter_context(
            tc.tile_pool(name="psum", bufs=1, space="PSUM"))

        ones16 = const_pool.tile([P, 1], f16)
        nc.vector.memset(ones16[:], 1.0)

        num_ps = psum_pool.tile([1, MMCHUNK], f32)
        den_ps = psum_pool.tile([1, MMCHUNK], f32)

        na = ablk

        # ---- DMA pre-issue (single ring; issue order = arrival order):
        # all seq tiles, then the LAST sub-block's val/rew (so its d/dm can
        # be computed ~20us early), then the val/rew/lp groups, and the last
        # sub-block's lp as the very final tile — its only remaining work is
        # the single mq op.
        seq_ts, lp_ts, val_ts, rew_ts = [], [], [], [None] * nsub
        val_ts = [None] * nsub
        lp_ts = [None] * nsub
        rew_ts = [None] * nsub
        for si in range(nsub):
            r0 = si * P * na
            seq_t = in_pool.tile([P, na, T], i32, tag="seq")
            nc.sync.dma_start(out=seq_t[:], in_=dram_sub(seq, r0, na))
            seq_ts.append(seq_t)
        li = nsub - 1

        # Optional second HWDGE ring (the ACT engine's): rew+lp stream there,
        # pre-issued ahead of all the casts on the same queue.
        ring2 = nc.scalar if bool(int(os.environ.get("K_RING_SPLIT", "0"))) \
            else nc.sync

        def issue_vr(si):
            r0 = si * P * na
            # NOTE: never slice the DRAM side along T — a strided DRAM
            # source defeats descriptor coalescing (~25 GB/s measured).
            val_ts[si] = in_pool.tile([P, na, T], f32, tag="val", name=f"val{si}")
            rew_ts[si] = in_pool.tile([P, na, T], f32, tag="rew", name=f"rew{si}")
            nc.sync.dma_start(out=val_ts[si][:], in_=dram_sub(val, r0, na))
            ring2.dma_start(out=rew_ts[si][:], in_=dram_sub(rew, r0, na))

        def issue_lp(si):
            r0 = si * P * na
            lp_ts[si] = in_pool.tile([P, na, T], f32, tag="lp", name=f"lp{si}")
            ring2.dma_start(out=lp_ts[si][:], in_=dram_sub(lp, r0, na))

        if nsub > 1:
            issue_vr(li)
        for si in range(nsub - 1):
            issue_vr(si)
            issue_lp(si)
        if nsub == 1:
            issue_vr(li)
        issue_lp(li)

        # ---- Phase 1: masks (scans) + den matmuls, hidden under the stream.
        # The scan with two 4-byte sources runs at half DVE rate (both read
        # ports consumed); casting seq to fp16 first (on the idle ACT
        # engine) doubles scan throughput.  i32 -> f16 is value-safe here:
        # seq ∈ [0, 10000) and any nonzero int stays nonzero in fp16, which
        # is all logical_and looks at.
        masks = []
        for si in range(nsub):
            seq_t = seq_ts[si]
            seq16 = h_pool.tile([P, na, T], f16, tag="seq16", bufs=nsub,
                                name=f"seq16_{si}")
            nc.scalar.copy(seq16[:], seq_t[:])
            mask = scr_pool.tile([P, na, T], f16, tag="mask", bufs=nsub)
            nc.vector.memset(mask[:, :, 0:1], 1.0)
            for a in range(na):
                nc.vector.tensor_tensor_scan(
                    out=mask[:, a, 1:T], data0=seq16[:, a, 0:T - 1],
                    data1=seq16[:, a, 0:T - 1], initial=1.0,
                    op0=Op.logical_and, op1=Op.bypass)
            for a in range(na):
                for c in range(0, T, MMCHUNK):
                    nc.tensor.matmul(
                        out=den_ps[:], lhsT=ones16[:],
                        rhs=mask[:, a, c:c + MMCHUNK],
                        start=(si == 0 and a == 0 and c == 0),
                        stop=(si == nsub - 1 and a == na - 1
                              and c == T - MMCHUNK))
            masks.append(mask)

        # ---- Phase 2: casts + d/dm/mq + num matmuls.
        # The last sub-block's d/dm are emitted FIRST (its val/rew streamed
        # right after the seqs, and it skips the casts — f32 at 1x), so that
        # when its lp lands as the stream's final tile, the one remaining op
        # is mq.  dm (not q=lp*d) is the grouping that makes the lp-
        # dependent work a single op.
        d_last = scr_pool.tile([P, na, T], f16, tag="d_last", bufs=1)
        nc.vector.tensor_tensor(out=d_last[:], in0=val_ts[li][:],
                                in1=rew_ts[li][:], op=Op.subtract)
        dm_last = scr_pool.tile([P, na, T], f16, tag="dm_last", bufs=1)
        nc.vector.tensor_tensor(out=dm_last[:], in0=d_last[:],
                                in1=masks[li][:], op=Op.mult)

        for si in range(nsub - 1):
            lp_t, val_t, rew_t = lp_ts[si], val_ts[si], rew_ts[si]
            mask = masks[si]
            lp16 = h_pool.tile([P, na, T], f16, tag="lp16")
            val16 = h_pool.tile([P, na, T], f16, tag="val16")
            rew16 = h_pool.tile([P, na, T], f16, tag="rew16")
            nc.scalar.copy(val16[:], val_t[:])
            nc.scalar.copy(rew16[:], rew_t[:])
            nc.scalar.copy(lp16[:], lp_t[:])

            d16 = scr_pool.tile([P, na, T], f16, tag="d")
            nc.vector.tensor_tensor(out=d16[:], in0=val16[:], in1=rew16[:],
                                    op=Op.subtract)
            dm = scr_pool.tile([P, na, T], f16, tag="dm")
            nc.vector.tensor_tensor(out=dm[:], in0=d16[:], in1=mask[:],
                                    op=Op.mult)
            mq = scr_pool.tile([P, na, T], f16, tag="mq", bufs=3)
            nc.vector.tensor_tensor(out=mq[:], in0=lp16[:], in1=dm[:],
                                    op=Op.mult)
            for a in range(na):
                for c in range(0, T, MMCHUNK):
                    nc.tensor.matmul(
                        out=num_ps[:], lhsT=ones16[:],
                        rhs=mq[:, a, c:c + MMCHUNK],
                        start=(si == 0 and a == 0 and c == 0),
                        stop=False)

        # Tail: the stream's final tile -> one TT -> two matmuls.
        mq_last = scr_pool.tile([P, na, T], f16, tag="mq_last", bufs=1)
        nc.vector.tensor_tensor(out=mq_last[:], in0=lp_ts[li][:],
                                in1=dm_last[:], op=Op.mult)
        for a in range(na):
            for c in range(0, T, MMCHUNK):
                nc.tensor.matmul(
                    out=num_ps[:], lhsT=ones16[:],
                    rhs=mq_last[:, a, c:c + MMCHUNK],
                    start=(nsub == 1 and a == 0 and c == 0),
                    stop=(a == na - 1 and c == T - MMCHUNK))

        # PSUM can't be DMA'd directly — bounce through SBUF.  den closed in
        # phase 1, so its copy + store fully overlap phase 2; num's copy is
        # split across the two free engines.
        num_sb = const_pool.tile([1, MMCHUNK], f32)
        den_sb = const_pool.tile([1, MMCHUNK], f32)
        nc.scalar.copy(den_sb[:], den_ps[:])
        nc.sync.dma_start(out=out_den[:], in_=den_sb[:])
        nc.vector.tensor_copy(num_sb[:], num_ps[:])
        nc.sync.dma_start(out=out_num[:], in_=num_sb[:])

    nc.finalize()
    return nc


def kernel(sample_seq, sample_seqLogprobs, sample_value, sample_reward):
    from concourse.bass_utils import run_bass_kernel_spmd

    seq = np.ascontiguousarray(np.asarray(sample_seq, dtype=np.int32))
    lp = np.ascontiguousarray(np.asarray(sample_seqLogprobs, dtype=np.float32))
    val = np.ascontiguousarray(np.asarray(sample_value, dtype=np.float32))
    rew = np.ascontiguousarray(np.asarray(sample_reward, dtype=np.float32))
    assert seq.shape == (B, T)

    if "nc" not in _CACHE:
        _CACHE["nc"] = _build_program(ROWS)
    nc = _CACHE["nc"]

    in_maps = []
    for c in range(NCORES):
        sl = slice(c * ROWS, (c + 1) * ROWS)
        in_maps.append({
            "seq": seq[sl], "lp": lp[sl], "val": val[sl], "rew": rew[sl],
        })

    trace = bool(int(os.environ.get("K_TRACE", "0")))
    res = run_bass_kernel_spmd(nc, in_maps, core_ids=list(range(NCORES)),
                               trace=trace)
    if trace:
        _CACHE["exec_time_ns"] = res.exec_time_ns
        _CACHE["trace"] = res.instructions_and_trace
    num = 0.0
    den = 0.0
    for r in res.results:
        num += float(np.asarray(r["out_num"], dtype=np.float64).sum())
        den += float(np.asarray(r["out_den"], dtype=np.float64).sum())
    return np.float32(num / den)
